# revision 1
# baseline (speedup 1.0000x reference)
"""Trainium2 Bass kernel for nn_Decoder_5111011083047 (moe_routing).

Decoder block: MoE-gated (4 experts, top-2) cross-attention QKV + exact-gelu MLP.
B=4096 tokens, DIM=2048, HIDDEN=8192, 4 heads of 512.

Strategy: data-parallel over tokens (512/core on 8 cores), weights replicated
(pre-cast bf16, pre-tiled on host). Activations live feature-major ([D, T]) on
device so every matmul contracts over the partition dim; per-token combine
weights are folded into 4 scaled copies of the activations and the expert sum
happens for free via PSUM accumulation. Gate runs in fp32 so top-2 selection
matches the reference. Only Q columns of expert_W are computed for ny and only
K/V columns for x (the reference computes all 3*DIM for both and discards).

SBUF is tight (224KB/partition, 4KB slot padding): long-lived tensors share
pool tags (slots) in sequential lifetime chains (xe->nye->hg, xTf->kvT->outv,
xTb->qT, yTb->esc, nyT->attnT, combT->rsbmsb) and small scratch is packed into
shared tiles.
"""

import numpy as np
import ml_dtypes

import concourse.bacc as bacc
import concourse.bass as bass
import concourse.tile as tile
from concourse import mybir
from concourse.bass_utils import run_bass_kernel_spmd
from concourse.masks import make_identity

F32 = mybir.dt.float32
BF16 = mybir.dt.bfloat16
NPBF16 = ml_dtypes.bfloat16

N_CORES = 8
B = 4096
TB = B // N_CORES  # 512 tokens per core
D = 2048
E = 4
H = 4
DH = D // H  # 512
HID = 4 * D  # 8192
KC = D // 128  # 16 chunks of the model dim
HC = HID // 128  # 64 chunks of the hidden dim
EPS = 1e-5

AF = mybir.ActivationFunctionType
ALU = mybir.AluOpType

TRACE = False
LAST_EXEC_NS = None
LAST_RESULTS = None


def build_program(gelu_func=AF.Gelu, reps: int = 1) -> bass.Bass:
    nc = bacc.Bacc(trn_type="TRN2")

    # ---- DRAM parameters (per-core shard + replicated weights) ----
    xTf = nc.declare_dram_parameter("xTf", [D, TB], F32, isOutput=False)
    xTb = nc.declare_dram_parameter("xTb", [D, TB], BF16, isOutput=False)
    yTb = nc.declare_dram_parameter("yTb", [D, TB], BF16, isOutput=False)
    yTf = nc.declare_dram_parameter("yTf", [D, TB], F32, isOutput=False)
    # packed fp32 consts, per-partition layout (see host side):
    # cols 0:4 gate_b | 4:8 expert_bias | 8:24 g1 | 24:40 b1 | 40:56 g2 | 56:72 b2
    # | 72:88 proj_b | 88:104 fc2_b | 104:168 fc1_b | 168:232 gate_W chunks
    cpack = nc.declare_dram_parameter("cpack", [128, 232], F32, isOutput=False)
    wkv = nc.declare_dram_parameter("wkv", [E, 2 * KC, 128, D], BF16, isOutput=False)
    wq = nc.declare_dram_parameter("wq", [E, KC, 128, D], BF16, isOutput=False)
    wproj = nc.declare_dram_parameter("wproj", [KC, 128, D], BF16, isOutput=False)
    wfc1 = nc.declare_dram_parameter("wfc1", [HC, 128, D], BF16, isOutput=False)
    wfc2 = nc.declare_dram_parameter("wfc2", [KC, 128, HID], BF16, isOutput=False)
    outT = nc.declare_dram_parameter("outT", [D, TB], F32, isOutput=True)

    with tile.TileContext(nc) as tc:
        with (
            tc.tile_pool(name="const", bufs=1) as constp,
            tc.tile_pool(name="big", bufs=1) as bigp,
            tc.tile_pool(name="tmp", bufs=2) as tmpp,
            tc.tile_pool(name="w", bufs=3) as wp,
            tc.tile_pool(name="mm", bufs=6, space="PSUM") as mmp,
            tc.tile_pool(name="sp", bufs=2, space="PSUM") as spp,
        ):
            # ---- constants ----
            cf = constp.tile([128, 232], F32)  # packed consts (DMA only)
            nc.gpsimd.dma_start(out=cf[:, 0:232], in_=cpack[:, :])
            gb_t = cf[:, 0:4]
            eb_t = cf[:, 4:8]
            g1_t = cf[:, 8:24]
            b1_t = cf[:, 24:40]
            g2_t = cf[:, 40:56]
            b2_t = cf[:, 56:72]
            pb_t = cf[:, 72:88]
            f2b_t = cf[:, 88:104]
            f1b_t = cf[:, 104:168]
            gw_t = cf[:, 168:232].rearrange("p (c e) -> p c e", e=E)
            cm = constp.tile([128, 132], F32)  # memset consts (DVE only)
            ones_row_f = cm[0:1, 0:128]
            nc.vector.memset(ones_row_f, 1.0)
            eps_t = cm[0:1, 128:129]
            nc.vector.memset(eps_t, EPS)
            cb = constp.tile([128, 132], BF16)
            ones_col_b = cb[:, 0:1]
            nc.vector.memset(ones_col_b, 1.0)
            ones_row_b = cb[0:1, 4:132]
            nc.vector.memset(ones_row_b, 1.0)
            ident = constp.tile([128, 128], F32)
            make_identity(nc, ident)

            def _emit_body():
                # ---- activations in ----
                xTf_t = bigp.tile([128, KC, TB], F32, tag="big32")  # slot: xTf->kvT->outv
                xTf_v = xTf.rearrange("(c p) t -> p c t", p=128)
                for c in range(KC):
                    nc.sync.dma_start(out=xTf_t[:, c, :], in_=xTf_v[:, c, :])
                xTb_t = bigp.tile([128, KC, TB], BF16, tag="big16a")  # slot: xTb->qT
                xTb_v = xTb.rearrange("(c p) t -> p c t", p=128)
                for c in range(KC):
                    nc.sync.dma_start(out=xTb_t[:, c, :], in_=xTb_v[:, c, :])
                yTb_t = bigp.tile([128, KC, TB], BF16, tag="big16b")  # slot: yTb->esc
                nc.sync.dma_start(out=yTb_t, in_=yTb.rearrange("(c p) t -> p c t", p=128))

                # ---- gate: fp32 logits, softmax, +expert_bias, top-2 combine ----
                combT = bigp.tile([1, E * TB], F32, tag="small8")  # expert e at [e*TB:]
                for tt in range(TB // 128):
                    gp = spp.tile([128, E], F32, tag="sp")
                    for c in range(KC):
                        nc.tensor.matmul(
                            gp,
                            lhsT=xTf_t[:, c, tt * 128 : (tt + 1) * 128],
                            rhs=gw_t[:, c, :],
                            start=(c == 0),
                            stop=(c == KC - 1),
                        )
                    # gate scratch packed in one tile:
                    # cols 0:4 p | 4:8 mask | 8:12 comb | 12 a | 13 b | 14 c | 15 d
                    # | 16 mn | 17 mx | 18 thr | 19 sumexp | 20 rinv
                    gs = tmpp.tile([128, 152], F32, tag="gs")
                    p_t = gs[:, 0:4]
                    nc.scalar.activation(
                        out=p_t, in_=gp, func=AF.Exp, accum_out=gs[:, 19:20]
                    )
                    nc.vector.reciprocal(gs[:, 20:21], gs[:, 19:20])
                    nc.vector.tensor_scalar_mul(p_t, p_t, gs[:, 20:21])
                    nc.vector.tensor_add(p_t, p_t, eb_t)
                    nc.vector.tensor_max(gs[:, 12:13], p_t[:, 0:1], p_t[:, 1:2])
                    nc.vector.tensor_tensor(gs[:, 13:14], p_t[:, 0:1], p_t[:, 1:2], op=ALU.min)
                    nc.vector.tensor_max(gs[:, 14:15], p_t[:, 2:3], p_t[:, 3:4])
                    nc.vector.tensor_tensor(gs[:, 15:16], p_t[:, 2:3], p_t[:, 3:4], op=ALU.min)
                    nc.vector.tensor_tensor(gs[:, 16:17], gs[:, 12:13], gs[:, 14:15], op=ALU.min)
                    nc.vector.tensor_max(gs[:, 17:18], gs[:, 13:14], gs[:, 15:16])
                    nc.vector.tensor_max(gs[:, 18:19], gs[:, 16:17], gs[:, 17:18])
                    nc.vector.tensor_scalar(
                        out=gs[:, 4:8], in0=p_t, scalar1=gs[:, 18:19], scalar2=None,
                        op0=ALU.is_ge,
                    )
                    # combine values into cols {24+32e}; transpose puts expert e
                    # on psum partition 32e (quadrant-aligned reads are legal)
                    for e in range(E):
                        nc.vector.tensor_mul(
                            gs[:, 24 + 32 * e : 25 + 32 * e],
                            p_t[:, e : e + 1],
                            gs[:, 4 + e : 5 + e],
                        )
                    ctp = spp.tile([128, 128], F32, tag="sp")
                    nc.tensor.transpose(ctp, gs[:, 24:152], ident)
                    for e in range(E):
                        nc.vector.tensor_copy(
                            combT[0:1, e * TB + tt * 128 : e * TB + (tt + 1) * 128],
                            ctp[32 * e : 32 * e + 1, :],
                        )

                # ---- broadcast combine rows across partitions (bf16) ----
                cbc = bigp.tile([128, E, TB], BF16, tag="cbc")
                for e in range(E):
                    bp = spp.tile([128, TB], F32, tag="sp")
                    nc.tensor.matmul(
                        bp,
                        lhsT=ones_row_f,
                        rhs=combT[0:1, e * TB : (e + 1) * TB],
                        start=True,
                        stop=True,
                    )
                    nc.vector.tensor_copy(cbc[:, e, :], bp)

                # ---- scaled copies of x per expert (DVE + gpsimd split so the
                # KV-gating chain finishes sooner; LN work is emitted after) ----
                xe = bigp.tile([128, E, KC, TB], BF16, tag="big64")  # slot: xe->nye->hg
                for e in range(E):
                    for c in range(KC):
                        eng = nc.gpsimd if (e * KC + c) % 3 == 2 else nc.vector
                        eng.tensor_mul(xe[:, e, c, :], xTb_t[:, c, :], cbc[:, e, :])

                # ---- layernorm stats over features (shared by both norms) ----
                ysum_p = spp.tile([1, TB], F32, tag="sp")
                ysq_p = spp.tile([1, TB], F32, tag="sp")
                for c in range(KC):
                    ysq = tmpp.tile([128, TB], BF16, tag="t1k")
                    nc.scalar.square(ysq, yTb_t[:, c, :])
                    nc.tensor.matmul(
                        ysum_p,
                        lhsT=ones_col_b,
                        rhs=yTb_t[:, c, :],
                        start=(c == 0),
                        stop=(c == KC - 1),
                    )
                    nc.tensor.matmul(
                        ysq_p, lhsT=ones_col_b, rhs=ysq, start=(c == 0), stop=(c == KC - 1)
                    )
                # stats scratch: st[0:1, 0:TB]=mean, [0:1, TB:2TB]=var->unused, [0:1,2TB:3TB]=rstd
                # later reused: st[0:4, 0:TB]=softmax sums, st[0:1, TB:3TB]=erow double-buffer
                st = tmpp.tile([4, 4 * TB], F32, tag="st", bufs=1)
                mean = st[0:1, 0:TB]
                var = st[0:1, TB : 2 * TB]
                rstd = st[0:1, 2 * TB : 3 * TB]
                nc.vector.tensor_scalar_mul(mean, ysum_p, 1.0 / D)
                nc.vector.tensor_mul(var, mean, mean)
                nc.vector.scalar_tensor_tensor(
                    out=var, in0=ysq_p, scalar=1.0 / D, in1=var, op0=ALU.mult,
                    op1=ALU.subtract,
                )
                nc.scalar.activation(rstd, var, func=AF.Sqrt, bias=eps_t)
                nc.vector.reciprocal(rstd, rstd)
                nc.vector.tensor_mul(mean, mean, rstd)  # mean <- mean*rstd
                rsbmsb = bigp.tile([128, 2, TB], BF16, tag="small8")
                rsb = rsbmsb[:, 0, :]
                msb = rsbmsb[:, 1, :]
                rsb_p = spp.tile([128, TB], F32, tag="sp")
                nc.tensor.matmul(rsb_p, lhsT=ones_row_f, rhs=rstd, start=True, stop=True)
                nc.vector.tensor_copy(rsb, rsb_p)
                msb_p = spp.tile([128, TB], F32, tag="sp")
                nc.tensor.matmul(msb_p, lhsT=ones_row_f, rhs=mean, start=True, stop=True)
                nc.vector.tensor_copy(msb, msb_p)

                # ---- normalized core + two affine variants ----
                nyT = bigp.tile([128, KC, TB], BF16, tag="big16c")  # slot: nyT->attnT
                hT = bigp.tile([128, KC, TB], BF16, tag="hT")
                for c in range(KC):
                    core = tmpp.tile([128, TB], BF16, tag="t1k")
                    nc.vector.tensor_mul(core, yTb_t[:, c, :], rsb)
                    nc.vector.tensor_sub(core, core, msb)
                    nc.vector.tensor_scalar(
                        out=nyT[:, c, :],
                        in0=core,
                        scalar1=g1_t[:, c : c + 1],
                        scalar2=b1_t[:, c : c + 1],
                        op0=ALU.mult,
                        op1=ALU.add,
                    )
                    nc.vector.tensor_scalar(
                        out=hT[:, c, :],
                        in0=core,
                        scalar1=g2_t[:, c : c + 1],
                        scalar2=b2_t[:, c : c + 1],
                        op0=ALU.mult,
                        op1=ALU.add,
                    )

                # ---- K,V matmuls (PSUM-accumulated over experts) ----
                kvT = bigp.tile([128, 2 * KC, TB], BF16, tag="big32")
                for m in range(2 * KC):
                    ps = mmp.tile([128, TB], F32, tag="mm")
                    for e in range(E):
                        wt = wp.tile([128, D], BF16, tag="w")
                        nc.sync.dma_start(out=wt, in_=wkv[e, m])
                        for k in range(KC):
                            nc.tensor.matmul(
                                ps,
                                lhsT=wt[:, k * 128 : (k + 1) * 128],
                                rhs=xe[:, e, k, :],
                                start=(e == 0 and k == 0),
                                stop=(e == E - 1 and k == KC - 1),
                            )
                    nc.scalar.copy(kvT[:, m, :], ps)

                # ---- scaled copies of ny; Q matmuls ----
                nye = bigp.tile([128, E, KC, TB], BF16, tag="big64")
                for e in range(E):
                    for c in range(KC):
                        eng = nc.gpsimd if (e * KC + c) % 3 == 2 else nc.vector
                        eng.tensor_mul(nye[:, e, c, :], nyT[:, c, :], cbc[:, e, :])
                qT = bigp.tile([128, KC, TB], BF16, tag="big16a")
                for m in range(KC):
                    ps = mmp.tile([128, TB], F32, tag="mm")
                    for e in range(E):
                        wt = wp.tile([128, D], BF16, tag="w")
                        nc.sync.dma_start(out=wt, in_=wq[e, m])
                        for k in range(KC):
                            nc.tensor.matmul(
                                ps,
                                lhsT=wt[:, k * 128 : (k + 1) * 128],
                                rhs=nye[:, e, k, :],
                                start=(e == 0 and k == 0),
                                stop=(e == E - 1 and k == KC - 1),
                            )
                    nc.scalar.copy(qT[:, m, :], ps)

                # ---- MLP fc1 + exact gelu (independent of attention; keeps PE busy) ----
                hg = bigp.tile([128, HC, TB], BF16, tag="big64")
                for m in range(HC):
                    ps = mmp.tile([128, TB], F32, tag="mm")
                    wt = wp.tile([128, D], BF16, tag="w")
                    nc.sync.dma_start(out=wt, in_=wfc1[m])
                    for k in range(KC):
                        nc.tensor.matmul(
                            ps,
                            lhsT=wt[:, k * 128 : (k + 1) * 128],
                            rhs=hT[:, k, :],
                            start=(k == 0),
                            stop=(k == KC - 1),
                        )
                    nc.scalar.activation(
                        out=hg[:, m, :], in_=ps, func=gelu_func, bias=f1b_t[:, m : m + 1]
                    )

                # ---- attention scores -> exp(.) rows, all on partition 0 ----
                esc = bigp.tile([1, H * H * TB], BF16, tag="big16b")
                scale = float(DH) ** -0.5
                for h in range(H):
                    for g in range(H):
                        sp_ = spp.tile([1, TB], F32, tag="sp")
                        for c2 in range(DH // 128):
                            pr = tmpp.tile([128, TB], BF16, tag="t1k")
                            nc.vector.tensor_mul(
                                pr, qT[:, h * 4 + c2, :], kvT[:, g * 4 + c2, :]
                            )
                            nc.tensor.matmul(
                                sp_,
                                lhsT=ones_col_b,
                                rhs=pr,
                                start=(c2 == 0),
                                stop=(c2 == DH // 128 - 1),
                            )
                        nc.scalar.activation(
                            out=esc[0:1, (h * H + g) * TB : (h * H + g + 1) * TB],
                            in_=sp_,
                            func=AF.Exp,
                            scale=scale,
                        )

                # ---- softmax sums over g (normalization folded into mixing) ----
                ssum = st[0:1, 0 : H * TB]
                nc.vector.tensor_reduce(
                    out=ssum.rearrange("p (h t) -> p h t", h=H),
                    in_=esc.rearrange("p (h g t) -> p h t g", h=H, g=H),
                    axis=mybir.AxisListType.X,
                    op=ALU.add,
                )
                nc.vector.reciprocal(ssum, ssum)

                # ---- mix V with attention weights (per query head) ----
                attnT = bigp.tile([128, KC, TB], BF16, tag="big16c")
                for h in range(H):
                    ebch = tmpp.tile([128, H, TB], BF16, tag="ebch", bufs=2)
                    for g in range(H):
                        bp = spp.tile([128, TB], F32, tag="sp")
                        nc.tensor.matmul(
                            bp,
                            lhsT=ones_row_b,
                            rhs=esc[0:1, (h * H + g) * TB : (h * H + g + 1) * TB],
                            start=True,
                            stop=True,
                        )
                        nc.vector.tensor_copy(ebch[:, g, :], bp)
                    rp = spp.tile([128, TB], F32, tag="sp")
                    nc.tensor.matmul(
                        rp,
                        lhsT=ones_row_f,
                        rhs=ssum[0:1, h * TB : (h + 1) * TB],
                        start=True,
                        stop=True,
                    )
                    rinvb = tmpp.tile([128, TB], BF16, tag="t1k")
                    nc.vector.tensor_copy(rinvb, rp)
                    for c2 in range(DH // 128):
                        acc = attnT[:, h * 4 + c2, :]
                        nc.vector.tensor_mul(acc, ebch[:, 0, :], kvT[:, KC + 0 * 4 + c2, :])
                        for g in range(1, H):
                            t2 = tmpp.tile([128, TB], BF16, tag="t1k")
                            nc.vector.tensor_mul(
                                t2, ebch[:, g, :], kvT[:, KC + g * 4 + c2, :]
                            )
                            nc.vector.tensor_add(acc, acc, t2)
                        nc.vector.tensor_mul(acc, acc, rinvb)

                # ---- out = y + fc2(gelu) + proj(attn) ----
                outv = bigp.tile([128, KC, TB], F32, tag="big32")
                nc.sync.dma_start(out=outv, in_=yTf.rearrange("(c p) t -> p c t", p=128))
                for m in range(KC):
                    ps = mmp.tile([128, TB], F32, tag="mm")
                    for quarter in range(4):
                        wt = wp.tile([128, D], BF16, tag="w")
                        nc.sync.dma_start(
                            out=wt, in_=wfc2[m][:, quarter * D : (quarter + 1) * D]
                        )
                        for kk in range(KC):
                            k = quarter * KC + kk
                            nc.tensor.matmul(
                                ps,
                                lhsT=wt[:, kk * 128 : (kk + 1) * 128],
                                rhs=hg[:, k, :],
                                start=(k == 0),
                                stop=(k == HC - 1),
                            )
                    nc.vector.scalar_tensor_tensor(
                        out=outv[:, m, :],
                        in0=ps,
                        scalar=f2b_t[:, m : m + 1],
                        in1=outv[:, m, :],
                        op0=ALU.add,
                        op1=ALU.add,
                    )
                for m in range(KC):
                    ps = mmp.tile([128, TB], F32, tag="mm")
                    wt = wp.tile([128, D], BF16, tag="w")
                    nc.sync.dma_start(out=wt, in_=wproj[m])
                    for k in range(KC):
                        nc.tensor.matmul(
                            ps,
                            lhsT=wt[:, k * 128 : (k + 1) * 128],
                            rhs=attnT[:, k, :],
                            start=(k == 0),
                            stop=(k == KC - 1),
                        )
                    nc.vector.scalar_tensor_tensor(
                        out=outv[:, m, :],
                        in0=ps,
                        scalar=pb_t[:, m : m + 1],
                        in1=outv[:, m, :],
                        op0=ALU.add,
                        op1=ALU.add,
                    )
                nc.sync.dma_start(out=outT.rearrange("(c p) t -> p c t", p=128), in_=outv)

            for _rep in range(reps):
                _emit_body()

    nc.compile()
    return nc

_cache: dict = {}


def _tile_w(w: np.ndarray) -> np.ndarray:
    """[K, F] -> [F//128, 128, K] tiles: out[m, p, k*128+f] = w[k*128+p, m*128+f]."""
    K, F = w.shape
    return np.ascontiguousarray(
        w.reshape(K // 128, 128, F // 128, 128)
        .transpose(2, 1, 0, 3)
        .reshape(F // 128, 128, K)
    )


def _prep_weights(inputs):
    bf = lambda a: np.ascontiguousarray(a).astype(NPBF16)
    expert_W = np.asarray(inputs["expert_W"], np.float32)
    wq = np.stack([_tile_w(expert_W[e, :, :D]) for e in range(E)])
    wkv = np.stack([_tile_w(expert_W[e, :, D:]) for e in range(E)])
    proj_W = np.asarray(inputs["proj_W"], np.float32)
    # attention output features are interleaved d*H+h; permute proj rows to h*DH+d
    projp = proj_W.reshape(DH, H, D).transpose(1, 0, 2).reshape(D, D)
    col = lambda v, n: np.asarray(v, np.float32).reshape(n, 128).T
    cpack = np.zeros((128, 232), np.float32)
    cpack[:, 0:4] = np.asarray(inputs["gate_b"], np.float32)[None, :]
    cpack[:, 4:8] = np.asarray(inputs["expert_bias"], np.float32)[None, :]
    cpack[:, 8:24] = col(inputs["norm1_g"], KC)
    cpack[:, 24:40] = col(inputs["norm1_b"], KC)
    cpack[:, 40:56] = col(inputs["norm2_g"], KC)
    cpack[:, 56:72] = col(inputs["norm2_b"], KC)
    cpack[:, 72:88] = col(inputs["proj_b"], KC)
    cpack[:, 88:104] = col(inputs["fc2_b"], KC)
    cpack[:, 104:168] = col(inputs["fc1_b"], HC)
    # gate_W chunks: [p, c*E + e] = gate_W[c*128+p, e]
    gwv = np.asarray(inputs["gate_W"], np.float32).reshape(KC, 128, E)
    cpack[:, 168:232] = gwv.transpose(1, 0, 2).reshape(128, KC * E)
    return {
        "cpack": np.ascontiguousarray(cpack),
        "wkv": bf(wkv),
        "wq": bf(wq),
        "wproj": bf(_tile_w(projp)),
        "wfc1": bf(_tile_w(np.asarray(inputs["fc1_W"], np.float32))),
        "wfc2": bf(_tile_w(np.asarray(inputs["fc2_W"], np.float32))),
    }


def _build_in_maps(inputs):
    x = np.asarray(inputs["x"], np.float32)
    y = np.asarray(inputs["y"], np.float32)
    shared = _prep_weights(inputs)
    in_maps = []
    for core in range(N_CORES):
        sl = slice(core * TB, (core + 1) * TB)
        xT = np.ascontiguousarray(x[sl].T)
        yT = np.ascontiguousarray(y[sl].T)
        m = {
            "xTf": xT,
            "xTb": xT.astype(NPBF16),
            "yTb": yT.astype(NPBF16),
            "yTf": yT,
        }
        m.update(shared)
        in_maps.append(m)
    return in_maps


def _get_program():
    if "nc" not in _cache:
        _cache["nc"] = build_program()
    return _cache["nc"]


def kernel(**inputs) -> np.ndarray:
    global LAST_EXEC_NS, LAST_RESULTS
    nc = _get_program()
    in_maps = _build_in_maps(inputs)
    res = run_bass_kernel_spmd(nc, in_maps, list(range(N_CORES)), trace=TRACE)
    LAST_EXEC_NS = res.exec_time_ns
    LAST_RESULTS = res
    out = np.concatenate(
        [np.asarray(res.results[i]["outT"]).T for i in range(N_CORES)], axis=0
    )
    return np.ascontiguousarray(out.astype(np.float32))


def _timed_exec(nc, in_maps, iters: int = 5):
    """Jit a held executable for nc; run `iters` times; return (outs, times).

    Mirrors bass2jax.run_bass_via_pjrt's multi-core branch, but keeps the
    jitted callable so iterations reuse the compiled NEFF.
    """
    import time

    import jax
    from jax.experimental.shard_map import shard_map
    from jax.sharding import Mesh, PartitionSpec

    from concourse import bass2jax, mybir as mb

    bass2jax.install_neuronx_cc_hook()

    partition_name = nc.partition_id_tensor.name if nc.partition_id_tensor else None
    in_names, out_names, out_avals, zero_outs = [], [], [], []
    for alloc in nc.m.functions[0].allocations:
        if not isinstance(alloc, mb.MemoryLocationSet):
            continue
        name = alloc.memorylocations[0].name
        if alloc.kind == "ExternalInput":
            if name != partition_name:
                in_names.append(name)
        elif alloc.kind == "ExternalOutput":
            out_names.append(name)
            shape = tuple(alloc.tensor_shape)
            dtype = mb.dt.np(alloc.dtype)
            out_avals.append(jax.core.ShapedArray(shape, dtype))
            zero_outs.append(np.zeros(shape, dtype))
    n_params = len(in_names)
    n_outs = len(out_avals)
    all_names = list(in_names) + list(out_names)
    if partition_name is not None:
        all_names.append(partition_name)

    def _body(*args):
        operands = list(args)
        if partition_name is not None:
            operands.append(bass2jax.partition_id_tensor())
        outs = bass2jax._bass_exec_p.bind(
            *operands,
            out_avals=tuple(out_avals),
            in_names=tuple(all_names),
            out_names=tuple(out_names),
            lowering_input_output_aliases=(),
            sim_require_finite=True,
            sim_require_nnan=True,
            nc=nc,
        )
        return tuple(outs)

    devices = jax.devices()[:N_CORES]
    mesh = Mesh(np.asarray(devices), ("core",))
    in_specs = (PartitionSpec("core"),) * (n_params + n_outs)
    out_specs = (PartitionSpec("core"),) * n_outs
    donate = tuple(range(n_params, n_params + n_outs))
    sharded = jax.jit(
        shard_map(
            _body, mesh=mesh, in_specs=in_specs, out_specs=out_specs, check_rep=False
        ),
        donate_argnums=donate,
        keep_unused=True,
    )
    concat_in = [
        np.concatenate(
            [np.asarray(in_maps[c][in_names[i]]) for c in range(N_CORES)], axis=0
        )
        for i in range(n_params)
    ]
    sharding = jax.sharding.NamedSharding(mesh, PartitionSpec("core"))
    dev_in = [jax.device_put(a, sharding) for a in concat_in]

    def zeros_dev():
        return [
            jax.device_put(
                np.zeros((N_CORES * z.shape[0], *z.shape[1:]), z.dtype), sharding
            )
            for z in zero_outs
        ]

    times = []
    out_arrs = None
    for _ in range(iters):
        zs = zeros_dev()
        jax.block_until_ready(zs)
        t0 = time.perf_counter()
        out_arrs = sharded(*dev_in, *zs)
        jax.block_until_ready(out_arrs)
        times.append(time.perf_counter() - t0)

    outs = {
        name: np.asarray(out_arrs[i]).reshape(N_CORES, *out_avals[i].shape)
        for i, name in enumerate(out_names)
    }
    return outs, times


def timed_run(inputs, iters: int = 5):
    """Returns (output [B, D] f32, per-iteration wall seconds)."""
    nc = _get_program()
    in_maps = _build_in_maps(inputs)
    outs, times = _timed_exec(nc, in_maps, iters)
    per_core = outs["outT"]
    out = np.concatenate([per_core[c].T for c in range(N_CORES)], axis=0)
    return np.ascontiguousarray(out.astype(np.float32)), times


def timed_chain(inputs, chain: int = 9, iters: int = 5):
    """Estimate per-execution device time via chained NEFF calls in one jit.

    Chains `chain` back-to-back kernel executions (outT of run k feeds yTf of
    run k+1, defeating CSE); compares against a 1-call jit. The slope
    (T_chain - T_1) / (chain - 1) cancels the axon dispatch overhead.
    Returns (times_chain, times_single) lists of wall seconds.
    """
    import time

    import jax
    import jax.numpy as jnp
    from jax.experimental.shard_map import shard_map
    from jax.sharding import Mesh, PartitionSpec

    from concourse import bass2jax, mybir as mb

    nc = _get_program()
    in_maps = _build_in_maps(inputs)
    bass2jax.install_neuronx_cc_hook()

    partition_name = nc.partition_id_tensor.name if nc.partition_id_tensor else None
    in_names, out_names, out_avals = [], [], []
    for alloc in nc.m.functions[0].allocations:
        if not isinstance(alloc, mb.MemoryLocationSet):
            continue
        name = alloc.memorylocations[0].name
        if alloc.kind == "ExternalInput":
            if name != partition_name:
                in_names.append(name)
        elif alloc.kind == "ExternalOutput":
            out_names.append(name)
            shape = tuple(alloc.tensor_shape)
            dtype = mb.dt.np(alloc.dtype)
            out_avals.append(jax.core.ShapedArray(shape, dtype))
    all_names = list(in_names) + list(out_names)
    if partition_name is not None:
        all_names.append(partition_name)
    yTf_idx = in_names.index("yTf")
    out_idx = out_names.index("outT")

    def _mk_body(n_calls):
        def _body(*args):
            ins = list(args)
            cur = None
            for _ in range(n_calls):
                ops = list(ins)
                if cur is not None:
                    ops[yTf_idx] = cur
                for av in out_avals:
                    ops.append(jnp.zeros(av.shape, av.dtype))
                if partition_name is not None:
                    ops.append(bass2jax.partition_id_tensor())
                outs = bass2jax._bass_exec_p.bind(
                    *ops,
                    out_avals=tuple(out_avals),
                    in_names=tuple(all_names),
                    out_names=tuple(out_names),
                    lowering_input_output_aliases=(),
                    sim_require_finite=True,
                    sim_require_nnan=True,
                    nc=nc,
                )
                cur = outs[out_idx]
            return (cur,)

        return _body

    devices = jax.devices()[:N_CORES]
    mesh = Mesh(np.asarray(devices), ("core",))
    n_params = len(in_names)
    in_specs = (PartitionSpec("core"),) * n_params
    out_specs = (PartitionSpec("core"),)
    concat_in = [
        np.concatenate(
            [np.asarray(in_maps[c][in_names[i]]) for c in range(N_CORES)], axis=0
        )
        for i in range(n_params)
    ]
    sharding = jax.sharding.NamedSharding(mesh, PartitionSpec("core"))
    dev_in = [jax.device_put(a, sharding) for a in concat_in]

    results = []
    for n_calls in (chain, 1):
        fn = jax.jit(
            shard_map(
                _mk_body(n_calls),
                mesh=mesh,
                in_specs=in_specs,
                out_specs=out_specs,
                check_rep=False,
            ),
            keep_unused=True,
        )
        out = fn(*dev_in)
        jax.block_until_ready(out)  # warm-up/compile
        ts = []
        for _ in range(iters):
            t0 = time.perf_counter()
            out = fn(*dev_in)
            jax.block_until_ready(out)
            ts.append(time.perf_counter() - t0)
        results.append(ts)
    return results[0], results[1]


def dispatch_floor(iters: int = 5):
    """Time a trivial 8-core kernel through the same path (dispatch overhead)."""
    import concourse.bacc as bacc2

    if "floor_nc" not in _cache:
        nc = bacc2.Bacc(trn_type="TRN2")
        a = nc.declare_dram_parameter("a", [128, 128], F32, isOutput=False)
        o = nc.declare_dram_parameter("o", [128, 128], F32, isOutput=True)
        with tile.TileContext(nc) as tc:
            with tc.tile_pool(name="s", bufs=1) as sp:
                at = sp.tile([128, 128], F32)
                nc.sync.dma_start(out=at, in_=a[:, :])
                nc.sync.dma_start(out=o[:, :], in_=at)
        nc.compile()
        _cache["floor_nc"] = nc
    arr = np.zeros((128, 128), np.float32)
    _, times = _timed_exec(_cache["floor_nc"], [{"a": arr}] * N_CORES, iters)
    return times



# revision 6
# speedup vs baseline: 3.6881x; 3.6881x over previous
"""Trainium2 Bass kernel for nn_Decoder_5111011083047 (moe_routing).

Decoder block: MoE-gated (4 experts, top-2) cross-attention QKV + exact-gelu MLP.
B=4096 tokens, DIM=2048, HIDDEN=8192, 4 heads of 512.

Strategy: data-parallel over tokens (512/core on 8 cores), weights replicated
(pre-tiled on host). Activations live feature-major ([D, T]) on device so every
matmul contracts over the partition dim; per-token combine weights are folded
into 4 scaled copies of the activations and the expert sum happens for free via
PSUM accumulation. Gate runs in fp32 so top-2 selection matches the reference.
Only Q columns of expert_W are computed for ny and only K/V columns for x.

The expert (KV, Q) and proj matmuls run in fp8e4m3 with DoubleRow perf mode
(two 128-row contraction slices per instruction -> 2x PE throughput); fc1/fc2
stay bf16 (the MLP path dominates the output scale, fp8 there overflows the
error budget; measured on host: fp8 on qkv+proj costs ~1.3e-2 rel err vs the
2e-2 gate, fp8 on fc1/fc2 would cost ~4e-2). Activations are scaled by SA=32
and weights by SW=1024 (pre-clipped to the 240 e4m3 max); PSUM copy-out
rescales by 2^-15.

SBUF is tight (224KB/partition, 4KB slot padding): long-lived tensors share
pool tags (slots) in sequential lifetime chains (xe->nye->hg, xTf->kvT->outv,
xTb->qT, yTb->esc, nyT->attnT, combT->rsbmsb) and small scratch is packed into
shared tiles.
"""

import numpy as np
import ml_dtypes

import concourse.bacc as bacc
import concourse.bass as bass
import concourse.tile as tile
from concourse import mybir
from concourse.bass_utils import run_bass_kernel_spmd
from concourse.masks import make_identity

F32 = mybir.dt.float32
BF16 = mybir.dt.bfloat16
FP8 = mybir.dt.float8e4
NPBF16 = ml_dtypes.bfloat16
NPFP8 = ml_dtypes.float8_e4m3

N_CORES = 8
B = 4096
TB = B // N_CORES  # 512 tokens per core
D = 2048
E = 4
H = 4
DH = D // H  # 512
HID = 4 * D  # 8192
KC = D // 128  # 16 chunks of the model dim
HC = HID // 128  # 64 chunks of the hidden dim
EPS = 1e-5

SA = 32.0  # fp8 activation scale
SW = 1024.0  # fp8 weight scale
RESCALE = 1.0 / (SA * SW)  # 2^-15, exact

AF = mybir.ActivationFunctionType
ALU = mybir.AluOpType
DR = mybir.MatmulPerfMode.DoubleRow

TRACE = False
LAST_EXEC_NS = None
LAST_RESULTS = None


def build_program(gelu_func=AF.Gelu, reps: int = 1) -> bass.Bass:
    nc = bacc.Bacc(trn_type="TRN2")

    # ---- DRAM parameters (per-core shard + replicated weights) ----
    xTf = nc.declare_dram_parameter("xTf", [D, TB], F32, isOutput=False)
    xTb = nc.declare_dram_parameter("xTb", [D, TB], BF16, isOutput=False)
    yTb = nc.declare_dram_parameter("yTb", [D, TB], BF16, isOutput=False)
    yTf = nc.declare_dram_parameter("yTf", [D, TB], F32, isOutput=False)
    # packed fp32 consts, per-partition layout (see host side):
    # cols 0:4 gate_b | 4:8 expert_bias | 8:24 g1 | 24:40 b1 | 40:56 g2 | 56:72 b2
    # | 72:88 proj_b | 88:104 fc2_b | 104:168 fc1_b | 168:232 gate_W chunks
    cpack = nc.declare_dram_parameter("cpack", [128, 232], F32, isOutput=False)
    wkv = nc.declare_dram_parameter("wkv", [E, 2 * KC, 128, D], FP8, isOutput=False)
    wq = nc.declare_dram_parameter("wq", [E, KC, 128, D], FP8, isOutput=False)
    wproj = nc.declare_dram_parameter("wproj", [KC, 128, D], FP8, isOutput=False)
    wfc1 = nc.declare_dram_parameter("wfc1", [HC, 128, D], BF16, isOutput=False)
    wfc2 = nc.declare_dram_parameter("wfc2", [KC, 128, HID], BF16, isOutput=False)
    outT = nc.declare_dram_parameter("outT", [D, TB], F32, isOutput=True)

    with tile.TileContext(nc) as tc:
        with (
            tc.tile_pool(name="const", bufs=1) as constp,
            tc.tile_pool(name="big", bufs=1) as bigp,
            tc.tile_pool(name="tmp", bufs=2) as tmpp,
            tc.tile_pool(name="w", bufs=3) as wp,
            tc.tile_pool(name="mm", bufs=6, space="PSUM") as mmp,
            tc.tile_pool(name="sp", bufs=2, space="PSUM") as spp,
        ):
            # ---- constants ----
            cf = constp.tile([128, 232], F32)  # packed consts (DMA only)
            nc.gpsimd.dma_start(out=cf[:, 0:232], in_=cpack[:, :])
            gb_t = cf[:, 0:4]
            eb_t = cf[:, 4:8]
            g1_t = cf[:, 8:24]
            b1_t = cf[:, 24:40]
            g2_t = cf[:, 40:56]
            b2_t = cf[:, 56:72]
            pb_t = cf[:, 72:88]
            f2b_t = cf[:, 88:104]
            f1b_t = cf[:, 104:168]
            gw_t = cf[:, 168:232].rearrange("p (c e) -> p c e", e=E)
            cm = constp.tile([128, 132], F32)  # memset consts (DVE only)
            ones_row_f = cm[0:1, 0:128]
            nc.vector.memset(ones_row_f, 1.0)
            eps_t = cm[0:1, 128:129]
            nc.vector.memset(eps_t, EPS)
            cb = constp.tile([128, 132], BF16)
            ones_col_b = cb[:, 0:1]
            nc.vector.memset(ones_col_b, 1.0)
            ones_row_b = cb[0:1, 4:132]
            nc.vector.memset(ones_row_b, 1.0)
            ident = constp.tile([128, 128], F32)
            make_identity(nc, ident)

            def _emit_body():
                # ---- activations in ----
                xTf_t = bigp.tile([128, KC, TB], F32, tag="big32")  # slot: xTf->kvT->outv
                xTf_v = xTf.rearrange("(c p) t -> p c t", p=128)
                for c in range(KC):
                    nc.sync.dma_start(out=xTf_t[:, c, :], in_=xTf_v[:, c, :])
                xTb_t = bigp.tile([128, KC, TB], BF16, tag="big16a")  # slot: xTb->qT
                xTb_v = xTb.rearrange("(c p) t -> p c t", p=128)
                for c in range(KC):
                    nc.sync.dma_start(out=xTb_t[:, c, :], in_=xTb_v[:, c, :])
                yTb_t = bigp.tile([128, KC, TB], BF16, tag="big16b")  # slot: yTb->esc
                nc.sync.dma_start(out=yTb_t, in_=yTb.rearrange("(c p) t -> p c t", p=128))

                # ---- gate: fp32 logits, softmax, +expert_bias, top-2 combine ----
                combT = bigp.tile([1, E * TB], F32, tag="small8")  # expert e at [e*TB:]
                for tt in range(TB // 128):
                    gp = spp.tile([128, E], F32, tag="sp")
                    for c in range(KC):
                        nc.tensor.matmul(
                            gp,
                            lhsT=xTf_t[:, c, tt * 128 : (tt + 1) * 128],
                            rhs=gw_t[:, c, :],
                            start=(c == 0),
                            stop=(c == KC - 1),
                        )
                    # gate scratch packed in one tile:
                    # cols 0:4 p | 4:8 mask | 8:12 comb | 12 a | 13 b | 14 c | 15 d
                    # | 16 mn | 17 mx | 18 thr | 19 sumexp | 20 rinv
                    gs = tmpp.tile([128, 152], F32, tag="gs")
                    p_t = gs[:, 0:4]
                    nc.scalar.activation(
                        out=p_t, in_=gp, func=AF.Exp, accum_out=gs[:, 19:20]
                    )
                    nc.vector.reciprocal(gs[:, 20:21], gs[:, 19:20])
                    nc.vector.tensor_scalar_mul(p_t, p_t, gs[:, 20:21])
                    nc.vector.tensor_add(p_t, p_t, eb_t)
                    nc.vector.tensor_max(gs[:, 12:13], p_t[:, 0:1], p_t[:, 1:2])
                    nc.vector.tensor_tensor(gs[:, 13:14], p_t[:, 0:1], p_t[:, 1:2], op=ALU.min)
                    nc.vector.tensor_max(gs[:, 14:15], p_t[:, 2:3], p_t[:, 3:4])
                    nc.vector.tensor_tensor(gs[:, 15:16], p_t[:, 2:3], p_t[:, 3:4], op=ALU.min)
                    nc.vector.tensor_tensor(gs[:, 16:17], gs[:, 12:13], gs[:, 14:15], op=ALU.min)
                    nc.vector.tensor_max(gs[:, 17:18], gs[:, 13:14], gs[:, 15:16])
                    nc.vector.tensor_max(gs[:, 18:19], gs[:, 16:17], gs[:, 17:18])
                    nc.vector.tensor_scalar(
                        out=gs[:, 4:8], in0=p_t, scalar1=gs[:, 18:19], scalar2=None,
                        op0=ALU.is_ge,
                    )
                    # combine values into cols {24+32e}; transpose puts expert e
                    # on psum partition 32e (quadrant-aligned reads are legal)
                    for e in range(E):
                        nc.vector.tensor_mul(
                            gs[:, 24 + 32 * e : 25 + 32 * e],
                            p_t[:, e : e + 1],
                            gs[:, 4 + e : 5 + e],
                        )
                    ctp = spp.tile([128, 128], F32, tag="sp")
                    nc.tensor.transpose(ctp, gs[:, 24:152], ident)
                    for e in range(E):
                        nc.vector.tensor_copy(
                            combT[0:1, e * TB + tt * 128 : e * TB + (tt + 1) * 128],
                            ctp[32 * e : 32 * e + 1, :],
                        )

                # ---- broadcast combine rows across partitions; fold in SA ----
                cbc = bigp.tile([128, E, TB], BF16, tag="cbc")
                for e in range(E):
                    bp = spp.tile([128, TB], F32, tag="sp")
                    nc.tensor.matmul(
                        bp,
                        lhsT=ones_row_f,
                        rhs=combT[0:1, e * TB : (e + 1) * TB],
                        start=True,
                        stop=True,
                    )
                    nc.vector.tensor_scalar_mul(cbc[:, e, :], bp, SA)

                # ---- fp8 scaled copies of x per expert (combine * SA folded in) ----
                xe = bigp.tile([128, E, KC, TB], FP8, tag="big64")  # slot: xe->nye->hg
                for e in range(E):
                    for c in range(KC):
                        nc.vector.tensor_mul(xe[:, e, c, :], xTb_t[:, c, :], cbc[:, e, :])

                # ---- layernorm stats over features (shared by both norms) ----
                ysum_p = spp.tile([1, TB], F32, tag="sp")
                ysq_p = spp.tile([1, TB], F32, tag="sp")
                for c in range(KC):
                    ysq = tmpp.tile([128, TB], BF16, tag="t1k")
                    nc.scalar.square(ysq, yTb_t[:, c, :])
                    nc.tensor.matmul(
                        ysum_p,
                        lhsT=ones_col_b,
                        rhs=yTb_t[:, c, :],
                        start=(c == 0),
                        stop=(c == KC - 1),
                    )
                    nc.tensor.matmul(
                        ysq_p, lhsT=ones_col_b, rhs=ysq, start=(c == 0), stop=(c == KC - 1)
                    )
                # stats scratch: st[0:1, 0:TB]=mean, [0:1, TB:2TB]=var->unused, [0:1,2TB:3TB]=rstd
                # later reused: st[0:4, 0:TB]=softmax sums, st[0:1, TB:3TB]=erow double-buffer
                st = tmpp.tile([4, 4 * TB], F32, tag="st", bufs=1)
                mean = st[0:1, 0:TB]
                var = st[0:1, TB : 2 * TB]
                rstd = st[0:1, 2 * TB : 3 * TB]
                nc.vector.tensor_scalar_mul(mean, ysum_p, 1.0 / D)
                nc.vector.tensor_mul(var, mean, mean)
                nc.vector.scalar_tensor_tensor(
                    out=var, in0=ysq_p, scalar=1.0 / D, in1=var, op0=ALU.mult,
                    op1=ALU.subtract,
                )
                nc.scalar.activation(rstd, var, func=AF.Sqrt, bias=eps_t)
                nc.vector.reciprocal(rstd, rstd)
                nc.vector.tensor_mul(mean, mean, rstd)  # mean <- mean*rstd
                rsbmsb = bigp.tile([128, 2, TB], BF16, tag="small8")
                rsb = rsbmsb[:, 0, :]
                msb = rsbmsb[:, 1, :]
                rsb_p = spp.tile([128, TB], F32, tag="sp")
                nc.tensor.matmul(rsb_p, lhsT=ones_row_f, rhs=rstd, start=True, stop=True)
                nc.vector.tensor_copy(rsb, rsb_p)
                msb_p = spp.tile([128, TB], F32, tag="sp")
                nc.tensor.matmul(msb_p, lhsT=ones_row_f, rhs=mean, start=True, stop=True)
                nc.vector.tensor_copy(msb, msb_p)

                # ---- normalized core + two affine variants ----
                nyT = bigp.tile([128, KC, TB], BF16, tag="big16c")  # slot: nyT->attnT
                hT = bigp.tile([128, KC, TB], BF16, tag="hT")
                for c in range(KC):
                    core = tmpp.tile([128, TB], BF16, tag="t1k")
                    nc.vector.tensor_mul(core, yTb_t[:, c, :], rsb)
                    nc.vector.tensor_sub(core, core, msb)
                    nc.vector.tensor_scalar(
                        out=nyT[:, c, :],
                        in0=core,
                        scalar1=g1_t[:, c : c + 1],
                        scalar2=b1_t[:, c : c + 1],
                        op0=ALU.mult,
                        op1=ALU.add,
                    )
                    nc.vector.tensor_scalar(
                        out=hT[:, c, :],
                        in0=core,
                        scalar1=g2_t[:, c : c + 1],
                        scalar2=b2_t[:, c : c + 1],
                        op0=ALU.mult,
                        op1=ALU.add,
                    )

                # ---- K,V matmuls: fp8 DoubleRow, PSUM-accumulated over experts ----
                kvT = bigp.tile([128, 2 * KC, TB], BF16, tag="big32")
                for m in range(2 * KC):
                    ps = mmp.tile([128, TB], F32, tag="mm")
                    for e in range(E):
                        wt = wp.tile([128, D], FP8, tag="w")
                        nc.sync.dma_start(out=wt, in_=wkv[e, m])
                        wtv = wt.rearrange("p (i j f) -> p i j f", j=2, f=128)
                        for i in range(KC // 2):
                            nc.tensor.matmul(
                                ps,
                                lhsT=wtv[:, i],
                                rhs=xe[:, e, 2 * i : 2 * i + 2, :],
                                start=(e == 0 and i == 0),
                                stop=(e == E - 1 and i == KC // 2 - 1),
                                perf_mode=DR,
                            )
                    nc.scalar.activation(
                        out=kvT[:, m, :], in_=ps, func=AF.Copy, scale=RESCALE
                    )

                # ---- fp8 scaled copies of ny; Q matmuls (fp8 DoubleRow) ----
                nye = bigp.tile([128, E, KC, TB], FP8, tag="big64")
                for e in range(E):
                    for c in range(KC):
                        nc.vector.tensor_mul(nye[:, e, c, :], nyT[:, c, :], cbc[:, e, :])
                qT = bigp.tile([128, KC, TB], BF16, tag="big16a")
                for m in range(KC):
                    ps = mmp.tile([128, TB], F32, tag="mm")
                    for e in range(E):
                        wt = wp.tile([128, D], FP8, tag="w")
                        nc.sync.dma_start(out=wt, in_=wq[e, m])
                        wtv = wt.rearrange("p (i j f) -> p i j f", j=2, f=128)
                        for i in range(KC // 2):
                            nc.tensor.matmul(
                                ps,
                                lhsT=wtv[:, i],
                                rhs=nye[:, e, 2 * i : 2 * i + 2, :],
                                start=(e == 0 and i == 0),
                                stop=(e == E - 1 and i == KC // 2 - 1),
                                perf_mode=DR,
                            )
                    nc.scalar.activation(
                        out=qT[:, m, :], in_=ps, func=AF.Copy, scale=RESCALE
                    )

                # ---- MLP fc1 + exact gelu (independent of attention; keeps PE busy) ----
                hg = bigp.tile([128, HC, TB], BF16, tag="big64")
                for m in range(HC):
                    ps = mmp.tile([128, TB], F32, tag="mm")
                    wt = wp.tile([128, D], BF16, tag="w")
                    nc.sync.dma_start(out=wt, in_=wfc1[m])
                    for k in range(KC):
                        nc.tensor.matmul(
                            ps,
                            lhsT=wt[:, k * 128 : (k + 1) * 128],
                            rhs=hT[:, k, :],
                            start=(k == 0),
                            stop=(k == KC - 1),
                        )
                    nc.scalar.activation(
                        out=hg[:, m, :], in_=ps, func=gelu_func, bias=f1b_t[:, m : m + 1]
                    )

                # ---- attention scores -> exp(.) rows, all on partition 0 ----
                esc = bigp.tile([1, H * H * TB], BF16, tag="big16b")
                scale = float(DH) ** -0.5
                for h in range(H):
                    for g in range(H):
                        sp_ = spp.tile([1, TB], F32, tag="sp")
                        for c2 in range(DH // 128):
                            pr = tmpp.tile([128, TB], BF16, tag="t1k")
                            nc.vector.tensor_mul(
                                pr, qT[:, h * 4 + c2, :], kvT[:, g * 4 + c2, :]
                            )
                            nc.tensor.matmul(
                                sp_,
                                lhsT=ones_col_b,
                                rhs=pr,
                                start=(c2 == 0),
                                stop=(c2 == DH // 128 - 1),
                            )
                        nc.scalar.activation(
                            out=esc[0:1, (h * H + g) * TB : (h * H + g + 1) * TB],
                            in_=sp_,
                            func=AF.Exp,
                            scale=scale,
                        )

                # ---- softmax sums over g (normalization folded into mixing) ----
                ssum = st[0:1, 0 : H * TB]
                nc.vector.tensor_reduce(
                    out=ssum.rearrange("p (h t) -> p h t", h=H),
                    in_=esc.rearrange("p (h g t) -> p h t g", h=H, g=H),
                    axis=mybir.AxisListType.X,
                    op=ALU.add,
                )
                nc.vector.reciprocal(ssum, ssum)

                # ---- mix V with attention weights (per query head) ----
                attnT = bigp.tile([128, KC, TB], BF16, tag="big16c")
                for h in range(H):
                    ebch = tmpp.tile([128, H, TB], BF16, tag="ebch", bufs=2)
                    for g in range(H):
                        bp = spp.tile([128, TB], F32, tag="sp")
                        nc.tensor.matmul(
                            bp,
                            lhsT=ones_row_b,
                            rhs=esc[0:1, (h * H + g) * TB : (h * H + g + 1) * TB],
                            start=True,
                            stop=True,
                        )
                        nc.vector.tensor_copy(ebch[:, g, :], bp)
                    rp = spp.tile([128, TB], F32, tag="sp")
                    nc.tensor.matmul(
                        rp,
                        lhsT=ones_row_f,
                        rhs=ssum[0:1, h * TB : (h + 1) * TB],
                        start=True,
                        stop=True,
                    )
                    rinvb = tmpp.tile([128, TB], BF16, tag="t1k")
                    nc.vector.tensor_copy(rinvb, rp)
                    for c2 in range(DH // 128):
                        acc = attnT[:, h * 4 + c2, :]
                        nc.vector.tensor_mul(acc, ebch[:, 0, :], kvT[:, KC + 0 * 4 + c2, :])
                        for g in range(1, H):
                            t2 = tmpp.tile([128, TB], BF16, tag="t1k")
                            nc.vector.tensor_mul(
                                t2, ebch[:, g, :], kvT[:, KC + g * 4 + c2, :]
                            )
                            nc.vector.tensor_add(acc, acc, t2)
                        nc.vector.tensor_mul(acc, acc, rinvb)

                # ---- quantize attnT to fp8 (reuses the dead esc slot) ----
                attn8 = bigp.tile([128, KC, TB], FP8, tag="big16b")
                for c in range(KC):
                    nc.vector.tensor_scalar_mul(attn8[:, c, :], attnT[:, c, :], SA)

                # ---- out = y + fc2(gelu) + proj(attn) ----
                outv = bigp.tile([128, KC, TB], F32, tag="big32")
                nc.sync.dma_start(out=outv, in_=yTf.rearrange("(c p) t -> p c t", p=128))
                for m in range(KC):
                    ps = mmp.tile([128, TB], F32, tag="mm")
                    for quarter in range(4):
                        wt = wp.tile([128, D], BF16, tag="w")
                        nc.sync.dma_start(
                            out=wt, in_=wfc2[m][:, quarter * D : (quarter + 1) * D]
                        )
                        for kk in range(KC):
                            k = quarter * KC + kk
                            nc.tensor.matmul(
                                ps,
                                lhsT=wt[:, kk * 128 : (kk + 1) * 128],
                                rhs=hg[:, k, :],
                                start=(k == 0),
                                stop=(k == HC - 1),
                            )
                    nc.vector.scalar_tensor_tensor(
                        out=outv[:, m, :],
                        in0=ps,
                        scalar=f2b_t[:, m : m + 1],
                        in1=outv[:, m, :],
                        op0=ALU.add,
                        op1=ALU.add,
                    )
                for m in range(KC):
                    ps = mmp.tile([128, TB], F32, tag="mm")
                    wt = wp.tile([128, D], FP8, tag="w")
                    nc.sync.dma_start(out=wt, in_=wproj[m])
                    wtv = wt.rearrange("p (i j f) -> p i j f", j=2, f=128)
                    for i in range(KC // 2):
                        nc.tensor.matmul(
                            ps,
                            lhsT=wtv[:, i],
                            rhs=attn8[:, 2 * i : 2 * i + 2, :],
                            start=(i == 0),
                            stop=(i == KC // 2 - 1),
                            perf_mode=DR,
                        )
                    # proj_b is pre-folded into f2b on the host, so outv += ps * 2^-15
                    nc.vector.scalar_tensor_tensor(
                        out=outv[:, m, :],
                        in0=ps,
                        scalar=RESCALE,
                        in1=outv[:, m, :],
                        op0=ALU.mult,
                        op1=ALU.add,
                    )
                nc.sync.dma_start(out=outT.rearrange("(c p) t -> p c t", p=128), in_=outv)

            for _rep in range(reps):
                _emit_body()

    nc.compile()
    return nc

_cache: dict = {}


def _tile_w(w: np.ndarray) -> np.ndarray:
    """[K, F] -> [F//128, 128, K] tiles: out[m, p, k*128+f] = w[k*128+p, m*128+f]."""
    K, F = w.shape
    return np.ascontiguousarray(
        w.reshape(K // 128, 128, F // 128, 128)
        .transpose(2, 1, 0, 3)
        .reshape(F // 128, 128, K)
    )


def _tile_w8(w: np.ndarray) -> np.ndarray:
    """[K, F] -> [F//128, 128, K] fp8 DoubleRow tiles.

    out[m, p, i*256 + j*128 + f] = q8(w[(2i+j)*128 + p, m*128 + f] * SW)
    so that a [128, 2, 128] slice at pair i is the DoubleRow lhsT for
    contraction rows [2i*128, (2i+2)*128).
    """
    K, F = w.shape
    q = np.clip(w.astype(np.float32) * SW, -240.0, 240.0).astype(NPFP8)
    a = q.reshape(K // 256, 2, 128, F // 128, 128)  # [i, j, p, m, f]
    return np.ascontiguousarray(
        a.transpose(3, 2, 0, 1, 4).reshape(F // 128, 128, K)
    )


def _prep_weights(inputs):
    bf = lambda a: np.ascontiguousarray(a).astype(NPBF16)
    expert_W = np.asarray(inputs["expert_W"], np.float32)
    wq = np.stack([_tile_w8(expert_W[e, :, :D]) for e in range(E)])
    wkv = np.stack([_tile_w8(expert_W[e, :, D:]) for e in range(E)])
    proj_W = np.asarray(inputs["proj_W"], np.float32)
    # attention output features are interleaved d*H+h; permute proj rows to h*DH+d
    projp = proj_W.reshape(DH, H, D).transpose(1, 0, 2).reshape(D, D)
    col = lambda v, n: np.asarray(v, np.float32).reshape(n, 128).T
    cpack = np.zeros((128, 232), np.float32)
    cpack[:, 0:4] = np.asarray(inputs["gate_b"], np.float32)[None, :]
    cpack[:, 4:8] = np.asarray(inputs["expert_bias"], np.float32)[None, :]
    cpack[:, 8:24] = col(inputs["norm1_g"], KC)
    cpack[:, 24:40] = col(inputs["norm1_b"], KC)
    cpack[:, 40:56] = col(inputs["norm2_g"], KC)
    cpack[:, 56:72] = col(inputs["norm2_b"], KC)
    cpack[:, 72:88] = col(inputs["proj_b"], KC)
    # proj_b folded into the fc2 bias column: outv = y + fc2out + (fc2_b+proj_b),
    # then the proj matmul adds only ps * RESCALE.
    cpack[:, 88:104] = col(
        np.asarray(inputs["fc2_b"], np.float32)
        + np.asarray(inputs["proj_b"], np.float32),
        KC,
    )
    cpack[:, 104:168] = col(inputs["fc1_b"], HC)
    # gate_W chunks: [p, c*E + e] = gate_W[c*128+p, e]
    gwv = np.asarray(inputs["gate_W"], np.float32).reshape(KC, 128, E)
    cpack[:, 168:232] = gwv.transpose(1, 0, 2).reshape(128, KC * E)
    return {
        "cpack": np.ascontiguousarray(cpack),
        "wkv": wkv,
        "wq": wq,
        "wproj": _tile_w8(projp),
        "wfc1": bf(_tile_w(np.asarray(inputs["fc1_W"], np.float32))),
        "wfc2": bf(_tile_w(np.asarray(inputs["fc2_W"], np.float32))),
    }


def _build_in_maps(inputs):
    x = np.asarray(inputs["x"], np.float32)
    y = np.asarray(inputs["y"], np.float32)
    shared = _prep_weights(inputs)
    in_maps = []
    for core in range(N_CORES):
        sl = slice(core * TB, (core + 1) * TB)
        xT = np.ascontiguousarray(x[sl].T)
        yT = np.ascontiguousarray(y[sl].T)
        m = {
            "xTf": xT,
            "xTb": xT.astype(NPBF16),
            "yTb": yT.astype(NPBF16),
            "yTf": yT,
        }
        m.update(shared)
        in_maps.append(m)
    return in_maps


def _get_program():
    if "nc" not in _cache:
        _cache["nc"] = build_program()
    return _cache["nc"]


def kernel(**inputs) -> np.ndarray:
    global LAST_EXEC_NS, LAST_RESULTS
    nc = _get_program()
    in_maps = _build_in_maps(inputs)
    res = run_bass_kernel_spmd(nc, in_maps, list(range(N_CORES)), trace=TRACE)
    LAST_EXEC_NS = res.exec_time_ns
    LAST_RESULTS = res
    out = np.concatenate(
        [np.asarray(res.results[i]["outT"]).T for i in range(N_CORES)], axis=0
    )
    return np.ascontiguousarray(out.astype(np.float32))


def make_runner(nc, in_maps, n_cores=N_CORES):
    """Jit a held 8-core executable for nc with device-resident inputs."""
    import jax
    from jax.experimental.shard_map import shard_map
    from jax.sharding import Mesh, PartitionSpec

    from concourse import bass2jax, mybir as mb

    bass2jax.install_neuronx_cc_hook()

    partition_name = nc.partition_id_tensor.name if nc.partition_id_tensor else None
    in_names, out_names, out_avals, zero_outs = [], [], [], []
    for alloc in nc.m.functions[0].allocations:
        if not isinstance(alloc, mb.MemoryLocationSet):
            continue
        name = alloc.memorylocations[0].name
        if alloc.kind == "ExternalInput":
            if name != partition_name:
                in_names.append(name)
        elif alloc.kind == "ExternalOutput":
            out_names.append(name)
            shape = tuple(alloc.tensor_shape)
            dtype = mb.dt.np(alloc.dtype)
            out_avals.append(jax.core.ShapedArray(shape, dtype))
            zero_outs.append(np.zeros(shape, dtype))
    n_params = len(in_names)
    all_names = list(in_names) + list(out_names)
    if partition_name is not None:
        all_names.append(partition_name)

    def _body(*args):
        operands = list(args)
        if partition_name is not None:
            operands.append(bass2jax.partition_id_tensor())
        outs = bass2jax._bass_exec_p.bind(
            *operands,
            out_avals=tuple(out_avals),
            in_names=tuple(all_names),
            out_names=tuple(out_names),
            lowering_input_output_aliases=(),
            sim_require_finite=True,
            sim_require_nnan=True,
            nc=nc,
        )
        return tuple(outs)

    devices = jax.devices()[:n_cores]
    mesh = Mesh(np.asarray(devices), ("core",))
    n_outs = len(out_avals)
    in_specs = (PartitionSpec("core"),) * (n_params + n_outs)
    out_specs = (PartitionSpec("core"),) * n_outs
    sharded = jax.jit(
        shard_map(
            _body, mesh=mesh, in_specs=in_specs, out_specs=out_specs, check_rep=False
        ),
        keep_unused=True,
    )
    concat_in = [
        np.concatenate(
            [np.asarray(in_maps[c][in_names[i]]) for c in range(n_cores)], axis=0
        )
        for i in range(n_params)
    ]
    sharding = jax.sharding.NamedSharding(mesh, PartitionSpec("core"))
    dev_in = [jax.device_put(a, sharding) for a in concat_in]
    dev_zeros = [
        jax.device_put(
            np.zeros((n_cores * z.shape[0], *z.shape[1:]), z.dtype), sharding
        )
        for z in zero_outs
    ]

    def run():
        import jax as _j

        out = sharded(*dev_in, *dev_zeros)
        _j.block_until_ready(out)
        return out

    run.out_names = out_names
    return run


def paired_slope(nc1, ncK, K, in_maps, iters=16, n_cores=N_CORES, verbose=True):
    """median over iters of (T_K - T_1)/(K-1), T_1/T_K timed back-to-back.

    Pairing cancels drift in the ~80ms axon dispatch overhead; the first two
    pairs are discarded as warm-up.
    """
    import time as _time

    r1 = make_runner(nc1, in_maps, n_cores)
    rK = make_runner(ncK, in_maps, n_cores)
    r1()
    rK()
    diffs = []
    for _ in range(iters + 2):
        t0 = _time.perf_counter()
        r1()
        t1 = _time.perf_counter()
        rK()
        t2 = _time.perf_counter()
        diffs.append(((t2 - t1) - (t1 - t0)) / (K - 1))
    diffs = np.array(diffs[2:])
    est = float(np.median(diffs))
    if verbose:
        print(
            f"  paired diffs ms: {np.array2string(diffs*1e3, precision=2, floatmode='fixed')}"
        )
        iqr = float(np.percentile(diffs, 75) - np.percentile(diffs, 25))
        print(f"  slope: {est*1e9:.0f} ns (IQR {iqr*1e9:.0f} ns)")
    return est * 1e9


def _timed_exec(nc, in_maps, iters: int = 5):
    """Legacy helper: run `iters` timed executions of one jitted program."""
    import time as _time

    run = make_runner(nc, in_maps)
    run()
    times = []
    out = None
    for _ in range(iters):
        t0 = _time.perf_counter()
        out = run()
        times.append(_time.perf_counter() - t0)
    return out, times


def dispatch_floor(iters: int = 5):
    """Time a trivial 8-core kernel through the same path (dispatch overhead)."""
    import concourse.bacc as bacc2

    if "floor_nc" not in _cache:
        nc = bacc2.Bacc(trn_type="TRN2")
        a = nc.declare_dram_parameter("a", [128, 128], F32, isOutput=False)
        o = nc.declare_dram_parameter("o", [128, 128], F32, isOutput=True)
        with tile.TileContext(nc) as tc:
            with tc.tile_pool(name="s", bufs=1) as sp:
                at = sp.tile([128, 128], F32)
                nc.sync.dma_start(out=at, in_=a[:, :])
                nc.sync.dma_start(out=o[:, :], in_=at)
        nc.compile()
        _cache["floor_nc"] = nc
    arr = np.zeros((128, 128), np.float32)
    _, times = _timed_exec(_cache["floor_nc"], [{"a": arr}] * N_CORES, iters)
    return times
